# revision 1
# baseline (speedup 1.0000x reference)
"""EnhancedGCN (LN -> GCNConv -> residual LN) as a Trainium2 Bass kernel.

Contract: kernel(**inputs) takes the full inputs from setup_inputs() and
returns the full [N, D] float32 output, running the compute on 8 axon
NeuronCores via run_bass_kernel_spmd.

Sharding: nodes are partitioned across the 8 cores by destination id
(graph/data parallel).  Each core computes h = dinv * LN(x) for all nodes
(replicated) into an HBM scratch, gathers h[src] rows for the edges whose
destination it owns (dma_gather, 512B rows), scatter-adds them into PSUM
accumulators with one-hot matmuls, then applies W, the dinv[dst] scale,
the residual and the second LN for its node range.  Params are replicated.
"""

import os
import sys

import numpy as np

for _p in ("/opt/trn_rl_repo", "/root/.axon_site/_ro/trn_rl_repo"):
    if os.path.isdir(_p) and _p not in sys.path:
        sys.path.insert(0, _p)

import bass_rust
import concourse.bacc as bacc
import concourse.bass as bass
import concourse.tile as tile
from concourse import mybir
from concourse.bass_utils import run_bass_kernel_spmd

# ---------------------------------------------------------------- constants
N = 100000
D = 128
EPS = 1e-5
NCORES = 8
NPAD = 100352                      # = 8 * 12544 = 8 * 98 * 128
PCN = NPAD // NCORES               # nodes per core (12544)
NBLK = PCN // 128                  # dst blocks per core (98)
NTILE = NPAD // 128                # x tiles (784)
WIN = 32768                        # gather window (int16 index range)
NWIN = (NPAD + WIN - 1) // WIN     # 4
PASS_BLOCKS = [20, 20, 20, 20, 18]
PASS_START = [0, 20, 40, 60, 80]
NPASS = len(PASS_BLOCKS)
CALL = 4096                        # edges per dma_gather call
CCH = CALL // 128                  # chunks per call (32)
T_CHUNK = 8                        # x tiles per phase-1 step

F32 = mybir.dt.float32
I16 = mybir.dt.int16
I32 = mybir.dt.int32


def _split_excess_waits(nc, max_waits=1):
    """walrus rejects >~2 sync waits per instruction; hoist overflow waits
    onto same-engine nops inserted before the instruction."""
    n = 0
    ctr = [0]
    for f in nc.m.functions:
        for bb in f.blocks:
            changed = False
            out = []
            for inst in bb.instructions:
                si = getattr(inst, "sync_info", None)
                waits = list(si.on_wait) if si is not None and si.on_wait else []
                if len(waits) > max_waits:
                    while len(waits) > max_waits:
                        take, waits = waits[:max_waits], waits[max_waits:]
                        ctr[0] += 1
                        nop = mybir.InstNoOp(
                            name=f"waitsplit-{ctr[0]}", ins=[], outs=[]
                        )
                        nop.engine = inst.engine
                        nop.sync_info = bass_rust.SyncInfo(
                            on_wait=take, on_update=[]
                        )
                        nc.register_instruction(nop)
                        out.append(nop)
                        n += 1
                    si.on_wait = waits
                    changed = True
                out.append(inst)
            if changed:
                bb.instructions = out
    return n


# ---------------------------------------------------------------- host prep
def _build_schedule(src, dst):
    """Partition + pad edges into the uniform per-core gather/matmul layout.

    Returns (sched, per_core) where sched is shared across cores and
    per_core holds the int16 idx/dstv arrays per core.
    """
    # order edges by (core, pass, window, block)
    core = dst // PCN
    blk = (dst % PCN) >> 7            # 0..97
    passid = np.minimum(blk // 20, NPASS - 1)
    win = src >> 15
    key = (((core * NPASS + passid) * NWIN + win) * NBLK + blk).astype(np.int64)
    order = np.argsort(key, kind="stable")
    s_src = src[order]
    s_dst = dst[order]
    s_key = key[order]

    cnt = np.bincount(s_key, minlength=NCORES * NPASS * NWIN * NBLK).reshape(
        NCORES, NPASS, NWIN, NBLK
    )

    # chunks per cell, uniform across cores
    pc = (cnt.max(axis=0) + 127) // 128          # [NPASS, NWIN, NBLK]
    for p in range(NPASS):
        b0, b1 = PASS_START[p], PASS_START[p] + PASS_BLOCKS[p]
        pc[p, :, :b0] = 0
        pc[p, :, b1:] = 0

    # global chunk layout: for (p, w): [cells b asc][tail pad chunks to CALL mult]
    chunk_block = []          # global chunk -> block id (pads -> a pass block)
    cell_chunk_start = np.zeros((NPASS, NWIN, NBLK), np.int64)
    calls = []                # (p, w, chunk_start, idx_col_start)
    for p in range(NPASS):
        b0, b1 = PASS_START[p], PASS_START[p] + PASS_BLOCKS[p]
        for w in range(NWIN):
            pw_start = len(chunk_block)
            for b in range(b0, b1):
                cell_chunk_start[p, w, b] = len(chunk_block)
                chunk_block.extend([b] * int(pc[p, w, b]))
            n_pw = len(chunk_block) - pw_start
            ncalls = (n_pw + CCH - 1) // CCH
            for k in range(ncalls):
                c0 = pw_start + k * CCH
                calls.append((p, w, c0, min(CCH, pw_start + n_pw - c0)))
    chunk_block = np.asarray(chunk_block, np.int64)
    tot_chunks = len(chunk_block)

    # start/stop flags: first/last chunk per block
    start_flag = np.zeros(tot_chunks, bool)
    stop_flag = np.zeros(tot_chunks, bool)
    first_seen = {}
    last_seen = {}
    for i, b in enumerate(chunk_block):
        if b not in first_seen:
            first_seen[b] = i
        last_seen[b] = i
    for b, i in first_seen.items():
        start_flag[i] = True
    for b, i in last_seen.items():
        stop_flag[i] = True

    # per-core slot arrays
    tot_slots = tot_chunks * 128
    per_core = []
    # per-edge slot: cell start + rank within (core, cell)
    cell_id = s_key  # unique per (core,p,w,b)
    # rank within cell
    cell_first = np.zeros_like(s_key)
    starts = np.searchsorted(s_key, np.arange(NCORES * NPASS * NWIN * NBLK))
    # ranks via grouped arange
    uniq, first_idx, counts = np.unique(s_key, return_index=True, return_counts=True)
    rank = np.arange(s_key.size) - np.repeat(first_idx, counts)
    pwb = s_key % (NPASS * NWIN * NBLK)
    pp = pwb // (NWIN * NBLK)
    ww = (pwb // NBLK) % NWIN
    bb = pwb % NBLK
    slot = cell_chunk_start[pp, ww, bb] * 128 + rank
    idxv = (s_src & (WIN - 1)).astype(np.int16)
    dstv = (s_dst & 127).astype(np.int16)
    edge_core = s_key // (NPASS * NWIN * NBLK)
    n_idx_cols = sum(cch * 8 for (_p, _w, _c0, cch) in calls)
    for c in range(NCORES):
        m = edge_core == c
        idx_arr = np.zeros(tot_slots, np.int16)
        dstv_arr = np.full(tot_slots, -1, np.int16)
        idx_arr[slot[m]] = idxv[m]
        dstv_arr[slot[m]] = dstv[m]
        blocks16 = []
        for (_cp, _cw, c0, cch) in calls:
            seg = idx_arr[c0 * 128:(c0 + cch) * 128]
            blocks16.append(np.tile(seg.reshape(-1, 16).T, (8, 1)))
        idx16 = np.concatenate(blocks16, axis=1)
        dstv16 = np.ascontiguousarray(dstv_arr.reshape(tot_chunks, 128).T)
        per_core.append((np.ascontiguousarray(idx16), dstv16))

    sched = {
        "chunk_block": chunk_block,
        "start": start_flag,
        "stop": stop_flag,
        "calls": calls,
        "tot_chunks": tot_chunks,
        "n_idx_cols": n_idx_cols,
    }
    return sched, per_core


# ------------------------------------------------------------ device program
def _build_program(sched, b1_nonzero, b_nonzero, g2_trivial, b2_zero, repeat=1):
    nc = bacc.Bacc("TRN2", target_bir_lowering=False, num_devices=NCORES,
                   num_swdge_queues=4)

    x_d = nc.dram_tensor("xin", [NPAD, D], F32, kind="ExternalInput")
    deg_d = nc.dram_tensor("deg", [128, NTILE], F32, kind="ExternalInput")
    degmy_d = nc.dram_tensor("degmy", [128, NBLK], F32, kind="ExternalInput")
    xres_d = nc.dram_tensor("xres", [PCN, D], F32, kind="ExternalInput")
    w_d = nc.dram_tensor("wmat", [D, D], F32, kind="ExternalInput")
    g1_d = nc.dram_tensor("g1v", [D, 1], F32, kind="ExternalInput")
    b1_d = nc.dram_tensor("b1v", [1, D], F32, kind="ExternalInput")
    g2_d = nc.dram_tensor("g2v", [1, D], F32, kind="ExternalInput")
    b2_d = nc.dram_tensor("b2v", [1, D], F32, kind="ExternalInput")
    bv_d = nc.dram_tensor("bvec", [1, D], F32, kind="ExternalInput")
    idx_d = nc.dram_tensor("idx16", [128, sched["n_idx_cols"]], I16, kind="ExternalInput")
    dstv_d = nc.dram_tensor("dstv", [128, sched["tot_chunks"]], I16, kind="ExternalInput")
    y_d = nc.dram_tensor("y", [PCN, D], F32, kind="ExternalOutput")

    chunk_block = sched["chunk_block"]
    start_flag = sched["start"]
    stop_flag = sched["stop"]
    calls = sched["calls"]
    call_idx_col = {}
    _c = 0
    for (_p, _w, _c0, _cch) in calls:
        call_idx_col[(_p, _w, _c0)] = _c
        _c += _cch * 8

    def bcast_row(dram):  # [1, D] dram -> partition-broadcast AP
        ap = dram[:]
        return bass.AP(tensor=ap.tensor, offset=ap.offset, ap=[[0, 128], [1, D]])

    with tile.TileContext(nc) as tc:
        with (
            tc.tile_pool(name="singles", bufs=1) as singles,
            tc.tile_pool(name="xin_p", bufs=3) as xin_p,
            tc.tile_pool(name="hout_p", bufs=3) as hout_p,
            tc.tile_pool(name="ph1s", bufs=4) as ph1s,
            tc.tile_pool(name="idx_p", bufs=2) as idx_p,
            tc.tile_pool(name="dstv_p", bufs=2) as dstv_p,
            tc.tile_pool(name="oh_p", bufs=2) as oh_p,
            tc.tile_pool(name="msg_p", bufs=2) as msg_p,
            tc.tile_pool(name="fin_p", bufs=4) as fin_p,
            tc.tile_pool(name="fins", bufs=8) as fins,
            tc.tile_pool(name="acc_ps", bufs=5, space="PSUM") as acc_ps,
            tc.tile_pool(name="mm_ps", bufs=2, space="PSUM") as mm_ps,
            tc.tile_pool(name="dram_p", bufs=1, space="DRAM") as dram_p,
        ):
            # ---------------- constants
            iota_i = singles.tile([128, 128], I32)
            nc.gpsimd.iota(iota_i[:], pattern=[[1, 128]], base=0, channel_multiplier=0)
            iota_f = singles.tile([128, 128], F32)
            nc.vector.tensor_copy(out=iota_f[:], in_=iota_i[:])

            w_sb = singles.tile([D, D], F32)
            nc.sync.dma_start(out=w_sb[:], in_=w_d[:])
            g1c = singles.tile([D, 1], F32)
            nc.sync.dma_start(out=g1c[:], in_=g1_d[:])
            # W' = g1[:,None] * W   (folds LN1 gamma into the weight matrix)
            wp_sb = singles.tile([D, D], F32)
            nc.vector.tensor_scalar_mul(out=wp_sb[:], in0=w_sb[:], scalar1=g1c[:])

            if b1_nonzero:
                b1_sb = singles.tile([128, D], F32)
                nc.sync.dma_start(out=b1_sb[:], in_=bcast_row(b1_d))
            if b_nonzero:
                bv_sb = singles.tile([128, D], F32)
                nc.sync.dma_start(out=bv_sb[:], in_=bcast_row(bv_d))
            if not g2_trivial:
                g2_sb = singles.tile([128, D], F32)
                nc.sync.dma_start(out=g2_sb[:], in_=bcast_row(g2_d))
            if not b2_zero:
                b2_sb = singles.tile([128, D], F32)
                nc.sync.dma_start(out=b2_sb[:], in_=bcast_row(b2_d))

            eps_sb = singles.tile([128, 1], F32)
            nc.vector.memset(eps_sb[:], EPS)
            zero512 = singles.tile([128, 512], F32)
            nc.vector.memset(zero512[:], 0.0)
            deg_sb = singles.tile([128, NTILE], F32)
            nc.sync.dma_start(out=deg_sb[:], in_=deg_d[:])
            dinv_sb = singles.tile([128, NTILE], F32)
            nc.scalar.activation(out=dinv_sb[:], in_=deg_sb[:],
                                 func=mybir.ActivationFunctionType.Sqrt)
            nc.vector.reciprocal(out=dinv_sb[:], in_=dinv_sb[:])

            degmy_sb = singles.tile([128, NBLK], F32)
            nc.sync.dma_start(out=degmy_sb[:], in_=degmy_d[:])
            dinvmy_sb = singles.tile([128, NBLK], F32)
            nc.scalar.activation(out=dinvmy_sb[:], in_=degmy_sb[:],
                                 func=mybir.ActivationFunctionType.Sqrt)
            nc.vector.reciprocal(out=dinvmy_sb[:], in_=dinvmy_sb[:])

            for rep in range(repeat):
                h_ws = [
                    dram_p.tile([min(WIN, NPAD - w * WIN), D], F32,
                                tag=f"hw{w}", name=f"h_w{w}_r{rep}")
                    for w in range(NWIN)
                ]
                x_view = x_d[:].rearrange("(t p) f -> p t f", p=128)
                h_views = [
                    h_ws[w][:].rearrange("(t p) f -> p t f", p=128)
                    for w in range(NWIN)
                ]
                TPW = WIN // 128  # x tiles per window (256)
                xres_view = xres_d[:].rearrange("(b p) f -> p b f", p=128)
                y_view = y_d[:].rearrange("(b p) f -> p b f", p=128)

                # ---------------- phase 1: h = dinv * (LN(x) * g1 (+ b1))
                inv_d = 1.0 / D
                for t0 in range(0, NTILE, T_CHUNK):
                    g = min(T_CHUNK, NTILE - t0)
                    xt = xin_p.tile([128, T_CHUNK, D], F32)
                    nc.sync.dma_start(out=xt[:, :g, :], in_=x_view[:, t0:t0 + g, :])
                    sum_t = ph1s.tile([128, T_CHUNK], F32)
                    nc.vector.tensor_reduce(
                        out=sum_t[:, :g], in_=xt[:, :g, :],
                        axis=mybir.AxisListType.X, op=mybir.AluOpType.add)
                    sq = xin_p.tile([128, T_CHUNK, D], F32, tag="sqtile")
                    nc.scalar.activation(out=sq[:, :g, :], in_=xt[:, :g, :],
                                         func=mybir.ActivationFunctionType.Square)
                    ssq_t = ph1s.tile([128, T_CHUNK], F32)
                    nc.vector.tensor_reduce(
                        out=ssq_t[:, :g], in_=sq[:, :g, :],
                        axis=mybir.AxisListType.X, op=mybir.AluOpType.add)
                    mu = ph1s.tile([128, T_CHUNK], F32)
                    nc.scalar.mul(out=mu[:, :g], in_=sum_t[:, :g], mul=inv_d)
                    var = ph1s.tile([128, T_CHUNK], F32)
                    nc.scalar.mul(out=var[:, :g], in_=ssq_t[:, :g], mul=inv_d)
                    mu2 = ph1s.tile([128, T_CHUNK], F32)
                    nc.vector.tensor_mul(out=mu2[:, :g], in0=mu[:, :g], in1=mu[:, :g])
                    nc.vector.tensor_sub(out=var[:, :g], in0=var[:, :g], in1=mu2[:, :g])
                    nc.scalar.activation(out=var[:, :g], in_=var[:, :g],
                                         func=mybir.ActivationFunctionType.Sqrt,
                                         bias=eps_sb[:])
                    rstd = ph1s.tile([128, T_CHUNK], F32)
                    nc.vector.reciprocal(out=rstd[:, :g], in_=var[:, :g])
                    # fold dinv into rstd; bias = -mu * rstd'
                    nc.vector.tensor_mul(out=rstd[:, :g], in0=rstd[:, :g],
                                         in1=dinv_sb[:, t0:t0 + g])
                    nmu = ph1s.tile([128, T_CHUNK], F32)
                    nc.vector.tensor_mul(out=nmu[:, :g], in0=mu[:, :g], in1=rstd[:, :g])
                    nc.scalar.mul(out=nmu[:, :g], in_=nmu[:, :g], mul=-1.0)
                    ht = hout_p.tile([128, T_CHUNK, D], F32)
                    nc.vector.tensor_tensor(
                        out=ht[:, :g, :], in0=xt[:, :g, :],
                        in1=rstd[:, :g].to_broadcast([128, g, D]),
                        op=mybir.AluOpType.mult)
                    nc.vector.tensor_tensor(
                        out=ht[:, :g, :], in0=ht[:, :g, :],
                        in1=nmu[:, :g].to_broadcast([128, g, D]),
                        op=mybir.AluOpType.add)
                    if b1_nonzero:
                        # h += dinv * b1  (rare path; b1 is zero in this problem)
                        for j in range(g):
                            tmp = ph1s.tile([128, D], F32, tag="b1tmp")
                            nc.vector.tensor_scalar_mul(
                                out=tmp[:], in0=b1_sb[:],
                                scalar1=dinv_sb[:, t0 + j:t0 + j + 1])
                            nc.vector.tensor_add(out=ht[:, j, :], in0=ht[:, j, :], in1=tmp[:])
                    wh = t0 // TPW
                    nc.sync.dma_start(
                        out=h_views[wh][:, t0 - wh * TPW:t0 - wh * TPW + g, :],
                        in_=ht[:, :g, :])

                # ---------------- phase 2: gather + one-hot matmul scatter
                idx_col = 0
                for p in range(NPASS):
                    accs = [
                        acc_ps.tile([128, 512], F32, tag="accbank", name=f"acc_r{rep}_p{p}_{i}")
                        for i in range(5)
                    ]
                    # start=True resets the whole PSUM bank, so initialize each
                    # bank exactly once and accumulate all chunks with start=False
                    for acc in accs:
                        nc.tensor.matmul(out=acc[:], lhsT=iota_f[:], rhs=zero512[:],
                                         start=True, stop=False, skip_group_check=True)
                    qrr = 0
                    for (cp, cw, chunk0, cch) in calls:
                        if cp != p:
                            if cp < p:
                                pass
                            continue
                        w = cw
                        S = cch * 8
                        ic0 = call_idx_col[(cp, cw, chunk0)]
                        idx_t = idx_p.tile([128, CCH * 8], I16)
                        nc.sync.dma_start(out=idx_t[:, :S], in_=idx_d[:, ic0:ic0 + S])
                        dstv_t = dstv_p.tile([128, CCH], I16)
                        nc.sync.dma_start(out=dstv_t[:, :cch], in_=dstv_d[:, chunk0:chunk0 + cch])
                        dstv_f = dstv_p.tile([128, CCH], F32, tag="dstvf")
                        nc.vector.tensor_copy(out=dstv_f[:, :cch], in_=dstv_t[:, :cch])
                        oh = oh_p.tile([128, CCH, 128], F32)
                        iap = iota_f[:]
                        nc.vector.tensor_tensor(
                            out=oh[:, :cch, :],
                            in0=dstv_f[:, :cch].to_broadcast([128, cch, 128]),
                            in1=bass.AP(tensor=iap.tensor, offset=iap.offset,
                                        ap=[list(iap.ap[0]), [0, cch], list(iap.ap[1])]),
                            op=mybir.AluOpType.is_equal)
                        msg = msg_p.tile([128, CCH, D], F32)
                        nc.gpsimd.dma_gather(
                            out_ap=msg[:, :cch, :],
                            in_ap=h_ws[w][:],
                            idxs_ap=idx_t[:, :S],
                            num_idxs=cch * 128, num_idxs_reg=cch * 128,
                            elem_size=D, single_packet=False,
                            queue_num=qrr % 4)
                        qrr += 1
                        for j in range(cch):
                            gi = chunk0 + j
                            b = int(chunk_block[gi])
                            a = b - PASS_START[p]
                            nc.tensor.matmul(
                                out=accs[a // 4][:, (a % 4) * 128:(a % 4) * 128 + 128],
                                lhsT=msg[:, j, :],
                                rhs=oh[:, j, :],
                                start=False,
                                stop=bool(stop_flag[gi]),
                                skip_group_check=True)

                    # ---------------- phase 3: finalize pass p (bank-batched)
                    p_end = PASS_START[p] + PASS_BLOCKS[p]
                    for b0 in range(PASS_START[p], p_end, 4):
                        g = min(4, p_end - b0)
                        bank = (b0 - PASS_START[p]) // 4
                        accS = fin_p.tile([128, 512], F32, tag="accS")
                        nc.vector.tensor_copy(out=accS[:, :g * 128],
                                              in_=accs[bank][:, :g * 128])
                        mm_bank = mm_ps.tile([128, 512], F32, tag="mmbank")
                        nc.tensor.matmul(out=mm_bank[:], lhsT=iota_f[:],
                                         rhs=zero512[:], start=True, stop=False,
                                         skip_group_check=True)
                        for j in range(g):
                            nc.tensor.matmul(
                                out=mm_bank[:, j * 128:(j + 1) * 128],
                                lhsT=accS[:, j * 128:(j + 1) * 128],
                                rhs=wp_sb[:], start=False, stop=True,
                                skip_group_check=True)
                        o1 = fin_p.tile([128, 4, 128], F32, tag="o1")
                        nc.vector.tensor_tensor(
                            out=o1[:, :g, :],
                            in0=mm_bank[:].rearrange("p (b f) -> p b f", b=4)[:, :g, :],
                            in1=dinvmy_sb[:, b0:b0 + g].to_broadcast([128, g, 128]),
                            op=mybir.AluOpType.mult)
                        if b_nonzero:
                            for j in range(g):
                                nc.vector.tensor_add(out=o1[:, j, :],
                                                     in0=o1[:, j, :], in1=bv_sb[:])
                        xr = fin_p.tile([128, 4, 128], F32, tag="xr")
                        nc.sync.dma_start(out=xr[:, :g, :],
                                          in_=xres_view[:, b0:b0 + g, :])
                        nc.vector.tensor_add(out=o1[:, :g, :], in0=o1[:, :g, :],
                                             in1=xr[:, :g, :])
                        # LN2, batched stats over g blocks
                        sum2 = fins.tile([128, 4], F32, tag="sum2")
                        nc.vector.tensor_reduce(out=sum2[:, :g], in_=o1[:, :g, :],
                                                axis=mybir.AxisListType.X,
                                                op=mybir.AluOpType.add)
                        sq2 = fin_p.tile([128, 4, 128], F32, tag="sq2")
                        nc.scalar.activation(out=sq2[:, :g, :], in_=o1[:, :g, :],
                                             func=mybir.ActivationFunctionType.Square)
                        ssq2 = fins.tile([128, 4], F32, tag="ssq2")
                        nc.vector.tensor_reduce(out=ssq2[:, :g], in_=sq2[:, :g, :],
                                                axis=mybir.AxisListType.X,
                                                op=mybir.AluOpType.add)
                        mu_2 = fins.tile([128, 4], F32, tag="mu2f")
                        nc.scalar.mul(out=mu_2[:, :g], in_=sum2[:, :g], mul=inv_d)
                        var2 = fins.tile([128, 4], F32, tag="var2")
                        nc.scalar.mul(out=var2[:, :g], in_=ssq2[:, :g], mul=inv_d)
                        mu22 = fins.tile([128, 4], F32, tag="mu22")
                        nc.vector.tensor_mul(out=mu22[:, :g], in0=mu_2[:, :g],
                                             in1=mu_2[:, :g])
                        nc.vector.tensor_sub(out=var2[:, :g], in0=var2[:, :g],
                                             in1=mu22[:, :g])
                        nc.scalar.activation(out=var2[:, :g], in_=var2[:, :g],
                                             func=mybir.ActivationFunctionType.Sqrt,
                                             bias=eps_sb[:])
                        rstd2 = fins.tile([128, 4], F32, tag="rstd2")
                        nc.vector.reciprocal(out=rstd2[:, :g], in_=var2[:, :g])
                        nmu2 = fins.tile([128, 4], F32, tag="nmu2")
                        nc.vector.tensor_mul(out=nmu2[:, :g], in0=mu_2[:, :g],
                                             in1=rstd2[:, :g])
                        nc.scalar.mul(out=nmu2[:, :g], in_=nmu2[:, :g], mul=-1.0)
                        yt = fin_p.tile([128, 4, 128], F32, tag="yt")
                        nc.vector.tensor_tensor(
                            out=yt[:, :g, :], in0=o1[:, :g, :],
                            in1=rstd2[:, :g].to_broadcast([128, g, 128]),
                            op=mybir.AluOpType.mult)
                        nc.vector.tensor_tensor(
                            out=yt[:, :g, :], in0=yt[:, :g, :],
                            in1=nmu2[:, :g].to_broadcast([128, g, 128]),
                            op=mybir.AluOpType.add)
                        if not g2_trivial:
                            for j in range(g):
                                nc.vector.tensor_mul(out=yt[:, j, :],
                                                     in0=yt[:, j, :], in1=g2_sb[:])
                        if not b2_zero:
                            for j in range(g):
                                nc.vector.tensor_add(out=yt[:, j, :],
                                                     in0=yt[:, j, :], in1=b2_sb[:])
                        nc.sync.dma_start(out=y_view[:, b0:b0 + g, :],
                                          in_=yt[:, :g, :])
    nc.compile()
    _split_excess_waits(nc, max_waits=1)
    return nc


_CACHE = {}


def _get_program_and_data(x, edge_index, W, b, g1, b1, g2, b2, repeat=1):
    src = np.asarray(edge_index[0], np.int64)
    dst = np.asarray(edge_index[1], np.int64)
    loops = np.arange(N, dtype=np.int64)
    src = np.concatenate([src, loops])
    dst = np.concatenate([dst, loops])

    deg = np.bincount(dst, minlength=NPAD).astype(np.float32)
    deg[deg == 0] = 1.0

    sched, per_core = _build_schedule(src, dst)

    b1_nonzero = bool(np.any(b1))
    b_nonzero = bool(np.any(b))
    g2_trivial = bool(np.all(g2 == 1.0))
    b2_zero = not bool(np.any(b2))

    nc = _build_program(sched, b1_nonzero, b_nonzero, g2_trivial, b2_zero, repeat=repeat)

    x_pad = np.zeros((NPAD, D), np.float32)
    x_pad[:N] = np.asarray(x, np.float32)
    deg_arr = np.ascontiguousarray(deg.reshape(NTILE, 128).T)

    in_maps = []
    for c in range(NCORES):
        idx16, dstv16 = per_core[c]
        degmy = np.ascontiguousarray(
            deg[c * PCN:(c + 1) * PCN].reshape(NBLK, 128).T)
        in_maps.append({
            "xin": x_pad,
            "deg": deg_arr,
            "degmy": degmy,
            "xres": x_pad[c * PCN:(c + 1) * PCN],
            "wmat": np.ascontiguousarray(np.asarray(W, np.float32)),
            "g1v": np.asarray(g1, np.float32).reshape(D, 1),
            "b1v": np.asarray(b1, np.float32).reshape(1, D),
            "g2v": np.asarray(g2, np.float32).reshape(1, D),
            "b2v": np.asarray(b2, np.float32).reshape(1, D),
            "bvec": np.asarray(b, np.float32).reshape(1, D),
            "idx16": idx16,
            "dstv": dstv16,
        })
    return nc, in_maps


def kernel(x, edge_index, W, b, g1, b1, g2, b2):
    nc, in_maps = _get_program_and_data(x, edge_index, W, b, g1, b1, g2, b2)
    res = run_bass_kernel_spmd(nc, in_maps, core_ids=list(range(NCORES)))
    out = np.concatenate([res.results[c]["y"] for c in range(NCORES)], axis=0)
    return out[:N].astype(np.float32)


if __name__ == "__main__":
    # tiny self-check of the host schedule against numpy
    rng = np.random.default_rng(0)
    E = 3200000
    src = rng.integers(0, N, E).astype(np.int64)
    dst = rng.integers(0, N, E).astype(np.int64)
    loops = np.arange(N, dtype=np.int64)
    srcl = np.concatenate([src, loops])
    dstl = np.concatenate([dst, loops])
    sched, per_core = _build_schedule(srcl, dstl)
    print("tot chunks:", sched["tot_chunks"], "calls:", len(sched["calls"]))
    print("pad overhead:", sched["tot_chunks"] * 128 / len(srcl) - 1)



# revision 11
# speedup vs baseline: 1.2500x; 1.2500x over previous
"""EnhancedGCN (LN -> GCNConv -> residual LN) as a Trainium2 Bass kernel.

Contract: kernel(**inputs) takes the full inputs from setup_inputs() and
returns the full [N, D] float32 output, running the compute on 8 axon
NeuronCores via run_bass_kernel_spmd.

Sharding: nodes are partitioned across the 8 cores by destination id
(graph/data parallel).  Each core computes h = dinv * LN(x) for all nodes
(replicated) into an HBM scratch, gathers h[src] rows for the edges whose
destination it owns (dma_gather, 512B rows), scatter-adds them into PSUM
accumulators with one-hot matmuls, then applies W, the dinv[dst] scale,
the residual and the second LN for its node range.  Params are replicated.
"""

import os
import sys

import numpy as np

for _p in ("/opt/trn_rl_repo", "/root/.axon_site/_ro/trn_rl_repo"):
    if os.path.isdir(_p) and _p not in sys.path:
        sys.path.insert(0, _p)

import bass_rust
import concourse.bacc as bacc
import concourse.bass as bass
import concourse.tile as tile
from concourse import mybir
from concourse.bass_utils import run_bass_kernel_spmd

# ---------------------------------------------------------------- constants
N = 100000
D = 128
EPS = 1e-5
NCORES = 8
NPAD = 100352                      # = 8 * 12544 = 8 * 98 * 128
PCN = NPAD // NCORES               # nodes per core (12544)
NBLK = PCN // 128                  # dst blocks per core (98)
NTILE = NPAD // 128                # x tiles (784)
WIN = 32768                        # gather window (int16 index range)
NWIN = (NPAD + WIN - 1) // WIN     # 4
PASS_BLOCKS = [20, 20, 20, 20, 18]
PASS_START = [0, 20, 40, 60, 80]
NPASS = len(PASS_BLOCKS)
CALL = 4096                        # edges per dma_gather call
CCH = CALL // 128                  # chunks per call (32)
T_CHUNK = 8                        # x tiles per phase-1 step

F32 = mybir.dt.float32
F16 = mybir.dt.float16
I16 = mybir.dt.int16
I32 = mybir.dt.int32


def _split_excess_waits(nc, max_waits=1):
    """walrus rejects >~2 sync waits per instruction; hoist overflow waits
    onto same-engine nops inserted before the instruction."""
    n = 0
    ctr = [0]
    for f in nc.m.functions:
        for bb in f.blocks:
            changed = False
            out = []
            for inst in bb.instructions:
                si = getattr(inst, "sync_info", None)
                waits = list(si.on_wait) if si is not None and si.on_wait else []
                if len(waits) > max_waits:
                    while len(waits) > max_waits:
                        take, waits = waits[:max_waits], waits[max_waits:]
                        ctr[0] += 1
                        nop = mybir.InstNoOp(
                            name=f"waitsplit-{ctr[0]}", ins=[], outs=[]
                        )
                        nop.engine = inst.engine
                        nop.sync_info = bass_rust.SyncInfo(
                            on_wait=take, on_update=[]
                        )
                        nc.register_instruction(nop)
                        out.append(nop)
                        n += 1
                    si.on_wait = waits
                    changed = True
                out.append(inst)
            if changed:
                bb.instructions = out
    return n


# ---------------------------------------------------------------- host prep
def _build_schedule(src, dst):
    """Partition + pad edges into the uniform per-core gather/matmul layout.

    Returns (sched, per_core) where sched is shared across cores and
    per_core holds the int16 idx/dstv arrays per core.
    """
    # order edges by (core, pass, window, block)
    core = dst // PCN
    blk = (dst % PCN) >> 7            # 0..97
    passid = np.minimum(blk // 20, NPASS - 1)
    win = src >> 15
    key = (((core * NPASS + passid) * NWIN + win) * NBLK + blk).astype(np.int64)
    order = np.argsort(key, kind="stable")
    s_src = src[order]
    s_dst = dst[order]
    s_key = key[order]

    cnt = np.bincount(s_key, minlength=NCORES * NPASS * NWIN * NBLK).reshape(
        NCORES, NPASS, NWIN, NBLK
    )

    # chunks per cell, uniform across cores
    pc = (cnt.max(axis=0) + 127) // 128          # [NPASS, NWIN, NBLK]
    for p in range(NPASS):
        b0, b1 = PASS_START[p], PASS_START[p] + PASS_BLOCKS[p]
        pc[p, :, :b0] = 0
        pc[p, :, b1:] = 0

    # global chunk layout: for (p, w): [cells b asc][tail pad chunks to CALL mult]
    chunk_block = []          # global chunk -> block id (pads -> a pass block)
    cell_chunk_start = np.zeros((NPASS, NWIN, NBLK), np.int64)
    calls = []                # (p, w, chunk_start, idx_col_start)
    for p in range(NPASS):
        b0, b1 = PASS_START[p], PASS_START[p] + PASS_BLOCKS[p]
        for w in range(NWIN):
            pw_start = len(chunk_block)
            for b in range(b0, b1):
                cell_chunk_start[p, w, b] = len(chunk_block)
                chunk_block.extend([b] * int(pc[p, w, b]))
            n_pw = len(chunk_block) - pw_start
            ncalls = (n_pw + CCH - 1) // CCH
            for k in range(ncalls):
                c0 = pw_start + k * CCH
                calls.append((p, w, c0, min(CCH, pw_start + n_pw - c0)))
    chunk_block = np.asarray(chunk_block, np.int64)
    tot_chunks = len(chunk_block)

    # start/stop flags: first/last chunk per block
    start_flag = np.zeros(tot_chunks, bool)
    stop_flag = np.zeros(tot_chunks, bool)
    first_seen = {}
    last_seen = {}
    for i, b in enumerate(chunk_block):
        if b not in first_seen:
            first_seen[b] = i
        last_seen[b] = i
    for b, i in first_seen.items():
        start_flag[i] = True
    for b, i in last_seen.items():
        stop_flag[i] = True

    # per-core slot arrays
    tot_slots = tot_chunks * 128
    per_core = []
    # per-edge slot: cell start + rank within (core, cell)
    cell_id = s_key  # unique per (core,p,w,b)
    # rank within cell
    cell_first = np.zeros_like(s_key)
    starts = np.searchsorted(s_key, np.arange(NCORES * NPASS * NWIN * NBLK))
    # ranks via grouped arange
    uniq, first_idx, counts = np.unique(s_key, return_index=True, return_counts=True)
    rank = np.arange(s_key.size) - np.repeat(first_idx, counts)
    pwb = s_key % (NPASS * NWIN * NBLK)
    pp = pwb // (NWIN * NBLK)
    ww = (pwb // NBLK) % NWIN
    bb = pwb % NBLK
    slot = cell_chunk_start[pp, ww, bb] * 128 + rank
    idxv = (s_src & (WIN - 1)).astype(np.int16)
    dstv = (s_dst & 127).astype(np.float16)
    edge_core = s_key // (NPASS * NWIN * NBLK)
    n_idx_cols = sum(cch * 8 for (_p, _w, _c0, cch) in calls)
    for c in range(NCORES):
        m = edge_core == c
        idx_arr = np.zeros(tot_slots, np.int16)
        dstv_arr = np.full(tot_slots, -1, np.float16)
        idx_arr[slot[m]] = idxv[m]
        dstv_arr[slot[m]] = dstv[m]
        blocks16 = []
        for (_cp, _cw, c0, cch) in calls:
            seg = idx_arr[c0 * 128:(c0 + cch) * 128]
            blocks16.append(np.tile(seg.reshape(-1, 16).T, (8, 1)))
        idx16 = np.concatenate(blocks16, axis=1)
        dstv16 = np.ascontiguousarray(dstv_arr.reshape(tot_chunks, 128).T)
        per_core.append((np.ascontiguousarray(idx16), dstv16))

    sched = {
        "chunk_block": chunk_block,
        "start": start_flag,
        "stop": stop_flag,
        "calls": calls,
        "tot_chunks": tot_chunks,
        "n_idx_cols": n_idx_cols,
    }
    return sched, per_core


# ------------------------------------------------------------ device program
def _build_program(sched, b1_nonzero, b_nonzero, g2_trivial, b2_zero, repeat=1):
    nc = bacc.Bacc("TRN2", target_bir_lowering=False, num_devices=NCORES,
                   num_swdge_queues=4)

    x_d = nc.dram_tensor("xin", [NPAD, D], F32, kind="ExternalInput")
    deg_d = nc.dram_tensor("deg", [128, NTILE], F32, kind="ExternalInput")
    degmy_d = nc.dram_tensor("degmy", [128, NBLK], F32, kind="ExternalInput")
    xres_d = nc.dram_tensor("xres", [PCN, D], F32, kind="ExternalInput")
    w_d = nc.dram_tensor("wmat", [D, D], F32, kind="ExternalInput")
    g1_d = nc.dram_tensor("g1v", [D, 1], F32, kind="ExternalInput")
    b1_d = nc.dram_tensor("b1v", [1, D], F32, kind="ExternalInput")
    g2_d = nc.dram_tensor("g2v", [1, D], F32, kind="ExternalInput")
    b2_d = nc.dram_tensor("b2v", [1, D], F32, kind="ExternalInput")
    bv_d = nc.dram_tensor("bvec", [1, D], F32, kind="ExternalInput")
    idx_d = nc.dram_tensor("idx16", [128, sched["n_idx_cols"]], I16, kind="ExternalInput")
    dstv_d = nc.dram_tensor("dstv", [128, sched["tot_chunks"]], F16, kind="ExternalInput")
    y_d = nc.dram_tensor("y", [PCN, D], F32, kind="ExternalOutput")

    chunk_block = sched["chunk_block"]
    start_flag = sched["start"]
    stop_flag = sched["stop"]
    calls = sched["calls"]
    call_idx_col = {}
    _c = 0
    for (_p, _w, _c0, _cch) in calls:
        call_idx_col[(_p, _w, _c0)] = _c
        _c += _cch * 8

    def bcast_row(dram):  # [1, D] dram -> partition-broadcast AP
        ap = dram[:]
        return bass.AP(tensor=ap.tensor, offset=ap.offset, ap=[[0, 128], [1, D]])

    with tile.TileContext(nc) as tc:
        with (
            tc.tile_pool(name="singles", bufs=1) as singles,
            tc.tile_pool(name="xin_p", bufs=3) as xin_p,
            tc.tile_pool(name="hout_p", bufs=3) as hout_p,
            tc.tile_pool(name="ph1s", bufs=4) as ph1s,
            tc.tile_pool(name="idx_p", bufs=3) as idx_p,
            tc.tile_pool(name="dstv_p", bufs=3) as dstv_p,
            tc.tile_pool(name="oh_p", bufs=3) as oh_p,
            tc.tile_pool(name="msg_p", bufs=3) as msg_p,
            tc.tile_pool(name="fin_p", bufs=4) as fin_p,
            tc.tile_pool(name="fins", bufs=8) as fins,
            tc.tile_pool(name="acc_ps", bufs=5, space="PSUM") as acc_ps,
            tc.tile_pool(name="mm_ps", bufs=2, space="PSUM") as mm_ps,
            tc.tile_pool(name="dram_p", bufs=1, space="DRAM") as dram_p,
        ):
            # ---------------- constants
            iota_i = singles.tile([128, 128], I32)
            nc.gpsimd.iota(iota_i[:], pattern=[[1, 128]], base=0, channel_multiplier=0)
            iota_f = singles.tile([128, 128], F32)
            nc.vector.tensor_copy(out=iota_f[:], in_=iota_i[:])
            iota_h = singles.tile([128, 128], F16)
            nc.vector.tensor_copy(out=iota_h[:], in_=iota_f[:])
            zero512h = singles.tile([128, 512], F16)
            nc.vector.memset(zero512h[:], 0.0)

            w_sb = singles.tile([D, D], F32)
            nc.sync.dma_start(out=w_sb[:], in_=w_d[:])
            g1c = singles.tile([D, 1], F32)
            nc.sync.dma_start(out=g1c[:], in_=g1_d[:])
            # W' = g1[:,None] * W   (folds LN1 gamma into the weight matrix)
            wp_sb = singles.tile([D, D], F32)
            nc.vector.tensor_scalar_mul(out=wp_sb[:], in0=w_sb[:], scalar1=g1c[:])

            if b1_nonzero:
                b1_sb = singles.tile([128, D], F32)
                nc.sync.dma_start(out=b1_sb[:], in_=bcast_row(b1_d))
            if b_nonzero:
                bv_sb = singles.tile([128, D], F32)
                nc.sync.dma_start(out=bv_sb[:], in_=bcast_row(bv_d))
            if not g2_trivial:
                g2_sb = singles.tile([128, D], F32)
                nc.sync.dma_start(out=g2_sb[:], in_=bcast_row(g2_d))
            if not b2_zero:
                b2_sb = singles.tile([128, D], F32)
                nc.sync.dma_start(out=b2_sb[:], in_=bcast_row(b2_d))

            eps_sb = singles.tile([128, 1], F32)
            nc.vector.memset(eps_sb[:], EPS)
            zero512 = singles.tile([128, 512], F32)
            nc.vector.memset(zero512[:], 0.0)
            deg_sb = singles.tile([128, NTILE], F32)
            nc.sync.dma_start(out=deg_sb[:], in_=deg_d[:])
            dinv_sb = singles.tile([128, NTILE], F32)
            nc.scalar.activation(out=dinv_sb[:], in_=deg_sb[:],
                                 func=mybir.ActivationFunctionType.Sqrt)
            nc.vector.reciprocal(out=dinv_sb[:], in_=dinv_sb[:])

            degmy_sb = singles.tile([128, NBLK], F32)
            nc.sync.dma_start(out=degmy_sb[:], in_=degmy_d[:])
            dinvmy_sb = singles.tile([128, NBLK], F32)
            nc.scalar.activation(out=dinvmy_sb[:], in_=degmy_sb[:],
                                 func=mybir.ActivationFunctionType.Sqrt)
            nc.vector.reciprocal(out=dinvmy_sb[:], in_=dinvmy_sb[:])

            for rep in range(repeat):
                h_ws = [
                    dram_p.tile([min(WIN, NPAD - w * WIN), D], F16,
                                tag=f"hw{w}", name=f"h_w{w}_r{rep}")
                    for w in range(NWIN)
                ]
                x_view = x_d[:].rearrange("(t p) f -> p t f", p=128)
                h_views = [
                    h_ws[w][:].rearrange("(t p) f -> p t f", p=128)
                    for w in range(NWIN)
                ]
                TPW = WIN // 128  # x tiles per window (256)
                xres_view = xres_d[:].rearrange("(b p) f -> p b f", p=128)
                y_view = y_d[:].rearrange("(b p) f -> p b f", p=128)

                # ---------------- phase 1: h = dinv * (LN(x) * g1 (+ b1))
                inv_d = 1.0 / D
                for t0 in range(0, NTILE, T_CHUNK):
                    g = min(T_CHUNK, NTILE - t0)
                    xt = xin_p.tile([128, T_CHUNK, D], F32)
                    nc.sync.dma_start(out=xt[:, :g, :], in_=x_view[:, t0:t0 + g, :])
                    sum_t = ph1s.tile([128, T_CHUNK], F32)
                    nc.vector.tensor_reduce(
                        out=sum_t[:, :g], in_=xt[:, :g, :],
                        axis=mybir.AxisListType.X, op=mybir.AluOpType.add)
                    sq = xin_p.tile([128, T_CHUNK, D], F32, tag="sqtile")
                    nc.scalar.activation(out=sq[:, :g, :], in_=xt[:, :g, :],
                                         func=mybir.ActivationFunctionType.Square)
                    ssq_t = ph1s.tile([128, T_CHUNK], F32)
                    nc.vector.tensor_reduce(
                        out=ssq_t[:, :g], in_=sq[:, :g, :],
                        axis=mybir.AxisListType.X, op=mybir.AluOpType.add)
                    mu = ph1s.tile([128, T_CHUNK], F32)
                    nc.scalar.mul(out=mu[:, :g], in_=sum_t[:, :g], mul=inv_d)
                    var = ph1s.tile([128, T_CHUNK], F32)
                    nc.scalar.mul(out=var[:, :g], in_=ssq_t[:, :g], mul=inv_d)
                    mu2 = ph1s.tile([128, T_CHUNK], F32)
                    nc.vector.tensor_mul(out=mu2[:, :g], in0=mu[:, :g], in1=mu[:, :g])
                    nc.vector.tensor_sub(out=var[:, :g], in0=var[:, :g], in1=mu2[:, :g])
                    nc.scalar.activation(out=var[:, :g], in_=var[:, :g],
                                         func=mybir.ActivationFunctionType.Sqrt,
                                         bias=eps_sb[:])
                    rstd = ph1s.tile([128, T_CHUNK], F32)
                    nc.vector.reciprocal(out=rstd[:, :g], in_=var[:, :g])
                    # fold dinv into rstd; bias = -mu * rstd'
                    nc.vector.tensor_mul(out=rstd[:, :g], in0=rstd[:, :g],
                                         in1=dinv_sb[:, t0:t0 + g])
                    nmu = ph1s.tile([128, T_CHUNK], F32)
                    nc.vector.tensor_mul(out=nmu[:, :g], in0=mu[:, :g], in1=rstd[:, :g])
                    nc.scalar.mul(out=nmu[:, :g], in_=nmu[:, :g], mul=-1.0)
                    htmp = hout_p.tile([128, T_CHUNK, D], F32, tag="htmp")
                    nc.vector.tensor_tensor(
                        out=htmp[:, :g, :], in0=xt[:, :g, :],
                        in1=rstd[:, :g].to_broadcast([128, g, D]),
                        op=mybir.AluOpType.mult)
                    ht = hout_p.tile([128, T_CHUNK, D], F16)
                    nc.vector.tensor_tensor(
                        out=ht[:, :g, :], in0=htmp[:, :g, :],
                        in1=nmu[:, :g].to_broadcast([128, g, D]),
                        op=mybir.AluOpType.add)
                    if b1_nonzero:
                        # h += dinv * b1  (rare path; b1 is zero in this problem)
                        for j in range(g):
                            tmp = ph1s.tile([128, D], F32, tag="b1tmp")
                            nc.vector.tensor_scalar_mul(
                                out=tmp[:], in0=b1_sb[:],
                                scalar1=dinv_sb[:, t0 + j:t0 + j + 1])
                            nc.vector.tensor_add(out=ht[:, j, :], in0=ht[:, j, :], in1=tmp[:])
                    wh = t0 // TPW
                    nc.sync.dma_start(
                        out=h_views[wh][:, t0 - wh * TPW:t0 - wh * TPW + g, :],
                        in_=ht[:, :g, :])

                # ---------------- phase 2: gather + one-hot matmul scatter
                idx_col = 0
                for p in range(NPASS):
                    accs = [
                        acc_ps.tile([128, 512], F32, tag="accbank", name=f"acc_r{rep}_p{p}_{i}")
                        for i in range(5)
                    ]
                    # start=True resets the whole PSUM bank, so initialize each
                    # bank exactly once and accumulate all chunks with start=False
                    for acc in accs:
                        nc.tensor.matmul(out=acc[:], lhsT=iota_h[:], rhs=zero512h[:],
                                         start=True, stop=False, skip_group_check=True)
                    qrr = 0
                    for (cp, cw, chunk0, cch) in calls:
                        if cp != p:
                            if cp < p:
                                pass
                            continue
                        w = cw
                        S = cch * 8
                        ic0 = call_idx_col[(cp, cw, chunk0)]
                        idx_t = idx_p.tile([128, CCH * 8], I16)
                        nc.sync.dma_start(out=idx_t[:, :S], in_=idx_d[:, ic0:ic0 + S])
                        dstv_t = dstv_p.tile([128, CCH], F16)
                        nc.sync.dma_start(out=dstv_t[:, :cch], in_=dstv_d[:, chunk0:chunk0 + cch])
                        oh = oh_p.tile([128, CCH, 128], F16)
                        iap = iota_h[:]
                        nc.vector.tensor_tensor(
                            out=oh[:, :cch, :],
                            in0=dstv_t[:, :cch].to_broadcast([128, cch, 128]),
                            in1=bass.AP(tensor=iap.tensor, offset=iap.offset,
                                        ap=[list(iap.ap[0]), [0, cch], list(iap.ap[1])]),
                            op=mybir.AluOpType.is_equal)
                        msg = msg_p.tile([128, CCH, D], F16)
                        nc.gpsimd.dma_gather(
                            out_ap=msg[:, :cch, :],
                            in_ap=h_ws[w][:],
                            idxs_ap=idx_t[:, :S],
                            num_idxs=cch * 128, num_idxs_reg=cch * 128,
                            elem_size=D, single_packet=False,
                            queue_num=qrr % 4)
                        qrr += 1
                        for j in range(cch):
                            gi = chunk0 + j
                            b = int(chunk_block[gi])
                            a = b - PASS_START[p]
                            nc.tensor.matmul(
                                out=accs[a // 4][:, (a % 4) * 128:(a % 4) * 128 + 128],
                                lhsT=msg[:, j, :],
                                rhs=oh[:, j, :],
                                start=False,
                                stop=bool(stop_flag[gi]),
                                skip_group_check=True)

                    # ---------------- phase 3: finalize pass p (bank-batched)
                    p_end = PASS_START[p] + PASS_BLOCKS[p]
                    for b0 in range(PASS_START[p], p_end, 4):
                        g = min(4, p_end - b0)
                        bank = (b0 - PASS_START[p]) // 4
                        accS = fin_p.tile([128, 512], F32, tag="accS")
                        nc.vector.tensor_copy(out=accS[:, :g * 128],
                                              in_=accs[bank][:, :g * 128])
                        mm_bank = mm_ps.tile([128, 512], F32, tag="mmbank")
                        nc.tensor.matmul(out=mm_bank[:], lhsT=iota_f[:],
                                         rhs=zero512[:], start=True, stop=False,
                                         skip_group_check=True)
                        for j in range(g):
                            nc.tensor.matmul(
                                out=mm_bank[:, j * 128:(j + 1) * 128],
                                lhsT=accS[:, j * 128:(j + 1) * 128],
                                rhs=wp_sb[:], start=False, stop=True,
                                skip_group_check=True)
                        o1 = fin_p.tile([128, 4, 128], F32, tag="o1")
                        nc.vector.tensor_tensor(
                            out=o1[:, :g, :],
                            in0=mm_bank[:].rearrange("p (b f) -> p b f", b=4)[:, :g, :],
                            in1=dinvmy_sb[:, b0:b0 + g].to_broadcast([128, g, 128]),
                            op=mybir.AluOpType.mult)
                        if b_nonzero:
                            for j in range(g):
                                nc.vector.tensor_add(out=o1[:, j, :],
                                                     in0=o1[:, j, :], in1=bv_sb[:])
                        xr = fin_p.tile([128, 4, 128], F32, tag="xr")
                        nc.sync.dma_start(out=xr[:, :g, :],
                                          in_=xres_view[:, b0:b0 + g, :])
                        nc.vector.tensor_add(out=o1[:, :g, :], in0=o1[:, :g, :],
                                             in1=xr[:, :g, :])
                        # LN2, batched stats over g blocks
                        sum2 = fins.tile([128, 4], F32, tag="sum2")
                        nc.vector.tensor_reduce(out=sum2[:, :g], in_=o1[:, :g, :],
                                                axis=mybir.AxisListType.X,
                                                op=mybir.AluOpType.add)
                        sq2 = fin_p.tile([128, 4, 128], F32, tag="sq2")
                        nc.scalar.activation(out=sq2[:, :g, :], in_=o1[:, :g, :],
                                             func=mybir.ActivationFunctionType.Square)
                        ssq2 = fins.tile([128, 4], F32, tag="ssq2")
                        nc.vector.tensor_reduce(out=ssq2[:, :g], in_=sq2[:, :g, :],
                                                axis=mybir.AxisListType.X,
                                                op=mybir.AluOpType.add)
                        mu_2 = fins.tile([128, 4], F32, tag="mu2f")
                        nc.scalar.mul(out=mu_2[:, :g], in_=sum2[:, :g], mul=inv_d)
                        var2 = fins.tile([128, 4], F32, tag="var2")
                        nc.scalar.mul(out=var2[:, :g], in_=ssq2[:, :g], mul=inv_d)
                        mu22 = fins.tile([128, 4], F32, tag="mu22")
                        nc.vector.tensor_mul(out=mu22[:, :g], in0=mu_2[:, :g],
                                             in1=mu_2[:, :g])
                        nc.vector.tensor_sub(out=var2[:, :g], in0=var2[:, :g],
                                             in1=mu22[:, :g])
                        nc.scalar.activation(out=var2[:, :g], in_=var2[:, :g],
                                             func=mybir.ActivationFunctionType.Sqrt,
                                             bias=eps_sb[:])
                        rstd2 = fins.tile([128, 4], F32, tag="rstd2")
                        nc.vector.reciprocal(out=rstd2[:, :g], in_=var2[:, :g])
                        nmu2 = fins.tile([128, 4], F32, tag="nmu2")
                        nc.vector.tensor_mul(out=nmu2[:, :g], in0=mu_2[:, :g],
                                             in1=rstd2[:, :g])
                        nc.scalar.mul(out=nmu2[:, :g], in_=nmu2[:, :g], mul=-1.0)
                        yt = fin_p.tile([128, 4, 128], F32, tag="yt")
                        nc.vector.tensor_tensor(
                            out=yt[:, :g, :], in0=o1[:, :g, :],
                            in1=rstd2[:, :g].to_broadcast([128, g, 128]),
                            op=mybir.AluOpType.mult)
                        nc.vector.tensor_tensor(
                            out=yt[:, :g, :], in0=yt[:, :g, :],
                            in1=nmu2[:, :g].to_broadcast([128, g, 128]),
                            op=mybir.AluOpType.add)
                        if not g2_trivial:
                            for j in range(g):
                                nc.vector.tensor_mul(out=yt[:, j, :],
                                                     in0=yt[:, j, :], in1=g2_sb[:])
                        if not b2_zero:
                            for j in range(g):
                                nc.vector.tensor_add(out=yt[:, j, :],
                                                     in0=yt[:, j, :], in1=b2_sb[:])
                        nc.sync.dma_start(out=y_view[:, b0:b0 + g, :],
                                          in_=yt[:, :g, :])
    nc.compile()
    _split_excess_waits(nc, max_waits=1)
    return nc


_CACHE = {}


def _get_program_and_data(x, edge_index, W, b, g1, b1, g2, b2, repeat=1):
    src = np.asarray(edge_index[0], np.int64)
    dst = np.asarray(edge_index[1], np.int64)
    loops = np.arange(N, dtype=np.int64)
    src = np.concatenate([src, loops])
    dst = np.concatenate([dst, loops])

    deg = np.bincount(dst, minlength=NPAD).astype(np.float32)
    deg[deg == 0] = 1.0

    sched, per_core = _build_schedule(src, dst)

    b1_nonzero = bool(np.any(b1))
    b_nonzero = bool(np.any(b))
    g2_trivial = bool(np.all(g2 == 1.0))
    b2_zero = not bool(np.any(b2))

    nc = _build_program(sched, b1_nonzero, b_nonzero, g2_trivial, b2_zero, repeat=repeat)

    x_pad = np.zeros((NPAD, D), np.float32)
    x_pad[:N] = np.asarray(x, np.float32)
    deg_arr = np.ascontiguousarray(deg.reshape(NTILE, 128).T)

    in_maps = []
    for c in range(NCORES):
        idx16, dstv16 = per_core[c]
        degmy = np.ascontiguousarray(
            deg[c * PCN:(c + 1) * PCN].reshape(NBLK, 128).T)
        in_maps.append({
            "xin": x_pad,
            "deg": deg_arr,
            "degmy": degmy,
            "xres": x_pad[c * PCN:(c + 1) * PCN],
            "wmat": np.ascontiguousarray(np.asarray(W, np.float32)),
            "g1v": np.asarray(g1, np.float32).reshape(D, 1),
            "b1v": np.asarray(b1, np.float32).reshape(1, D),
            "g2v": np.asarray(g2, np.float32).reshape(1, D),
            "b2v": np.asarray(b2, np.float32).reshape(1, D),
            "bvec": np.asarray(b, np.float32).reshape(1, D),
            "idx16": idx16,
            "dstv": dstv16,
        })
    return nc, in_maps


def kernel(x, edge_index, W, b, g1, b1, g2, b2):
    nc, in_maps = _get_program_and_data(x, edge_index, W, b, g1, b1, g2, b2)
    res = run_bass_kernel_spmd(nc, in_maps, core_ids=list(range(NCORES)))
    out = np.concatenate([res.results[c]["y"] for c in range(NCORES)], axis=0)
    return out[:N].astype(np.float32)


if __name__ == "__main__":
    # tiny self-check of the host schedule against numpy
    rng = np.random.default_rng(0)
    E = 3200000
    src = rng.integers(0, N, E).astype(np.int64)
    dst = rng.integers(0, N, E).astype(np.int64)
    loops = np.arange(N, dtype=np.int64)
    srcl = np.concatenate([src, loops])
    dstl = np.concatenate([dst, loops])
    sched, per_core = _build_schedule(srcl, dstl)
    print("tot chunks:", sched["tot_chunks"], "calls:", len(sched["calls"]))
    print("pad overhead:", sched["tot_chunks"] * 128 / len(srcl) - 1)



# revision 12
# speedup vs baseline: 1.6609x; 1.3287x over previous
"""EnhancedGCN (LN -> GCNConv -> residual LN) as a Trainium2 Bass kernel.

Contract: kernel(**inputs) takes the full inputs from setup_inputs() and
returns the full [N, D] float32 output, running the compute on 8 axon
NeuronCores via run_bass_kernel_spmd.

Sharding: nodes are partitioned across the 8 cores by destination id
(graph/data parallel).  Each core computes h = dinv * LN(x) for all nodes
(replicated) into an HBM scratch, gathers h[src] rows for the edges whose
destination it owns (dma_gather, 512B rows), scatter-adds them into PSUM
accumulators with one-hot matmuls, then applies W, the dinv[dst] scale,
the residual and the second LN for its node range.  Params are replicated.
"""

import os
import sys

import numpy as np

for _p in ("/opt/trn_rl_repo", "/root/.axon_site/_ro/trn_rl_repo"):
    if os.path.isdir(_p) and _p not in sys.path:
        sys.path.insert(0, _p)

import bass_rust
import concourse.bacc as bacc
import concourse.bass as bass
import concourse.tile as tile
from concourse import mybir
from concourse.bass_utils import run_bass_kernel_spmd

# ---------------------------------------------------------------- constants
N = 100000
D = 128
EPS = 1e-5
NCORES = 8
NPAD = 100352                      # = 8 * 12544 = 8 * 98 * 128
PCN = NPAD // NCORES               # nodes per core (12544)
NBLK = PCN // 128                  # dst blocks per core (98)
NTILE = NPAD // 128                # x tiles (784)
WIN = 32768                        # gather window (int16 index range)
NWIN = (NPAD + WIN - 1) // WIN     # 4
PASS_BLOCKS = [20, 20, 20, 20, 18]
PASS_START = [0, 20, 40, 60, 80]
NPASS = len(PASS_BLOCKS)
CALL = 4096                        # edges per dma_gather call
CCH = CALL // 128                  # chunks per call (32)
T_CHUNK = 8                        # x tiles per phase-1 step

F32 = mybir.dt.float32
F16 = mybir.dt.float16
I16 = mybir.dt.int16
I32 = mybir.dt.int32


def _split_excess_waits(nc, max_waits=1):
    """walrus rejects >~2 sync waits per instruction; hoist overflow waits
    onto same-engine nops inserted before the instruction."""
    n = 0
    ctr = [0]
    for f in nc.m.functions:
        for bb in f.blocks:
            changed = False
            out = []
            for inst in bb.instructions:
                si = getattr(inst, "sync_info", None)
                waits = list(si.on_wait) if si is not None and si.on_wait else []
                if len(waits) > max_waits:
                    while len(waits) > max_waits:
                        take, waits = waits[:max_waits], waits[max_waits:]
                        ctr[0] += 1
                        nop = mybir.InstNoOp(
                            name=f"waitsplit-{ctr[0]}", ins=[], outs=[]
                        )
                        nop.engine = inst.engine
                        nop.sync_info = bass_rust.SyncInfo(
                            on_wait=take, on_update=[]
                        )
                        nc.register_instruction(nop)
                        out.append(nop)
                        n += 1
                    si.on_wait = waits
                    changed = True
                out.append(inst)
            if changed:
                bb.instructions = out
    return n


# ---------------------------------------------------------------- host prep
def _build_schedule(src, dst):
    """Partition + pad edges into the uniform per-core gather/matmul layout.

    Returns (sched, per_core) where sched is shared across cores and
    per_core holds the int16 idx/dstv arrays per core.
    """
    # order edges by (core, pass, window, block)
    core = dst // PCN
    blk = (dst % PCN) >> 7            # 0..97
    passid = np.minimum(blk // 20, NPASS - 1)
    win = src >> 15
    key = (((core * NPASS + passid) * NWIN + win) * NBLK + blk).astype(np.int64)
    order = np.argsort(key, kind="stable")
    s_src = src[order]
    s_dst = dst[order]
    s_key = key[order]

    cnt = np.bincount(s_key, minlength=NCORES * NPASS * NWIN * NBLK).reshape(
        NCORES, NPASS, NWIN, NBLK
    )

    # chunks per cell, uniform across cores
    pc = (cnt.max(axis=0) + 127) // 128          # [NPASS, NWIN, NBLK]
    for p in range(NPASS):
        b0, b1 = PASS_START[p], PASS_START[p] + PASS_BLOCKS[p]
        pc[p, :, :b0] = 0
        pc[p, :, b1:] = 0

    # global chunk layout: for (p, w): [cells b asc][tail pad chunks to CALL mult]
    chunk_block = []          # global chunk -> block id (pads -> a pass block)
    cell_chunk_start = np.zeros((NPASS, NWIN, NBLK), np.int64)
    calls = []                # (p, w, chunk_start, idx_col_start)
    for p in range(NPASS):
        b0, b1 = PASS_START[p], PASS_START[p] + PASS_BLOCKS[p]
        for w in range(NWIN):
            pw_start = len(chunk_block)
            for b in range(b0, b1):
                cell_chunk_start[p, w, b] = len(chunk_block)
                chunk_block.extend([b] * int(pc[p, w, b]))
            n_pw = len(chunk_block) - pw_start
            ncalls = (n_pw + CCH - 1) // CCH
            for k in range(ncalls):
                c0 = pw_start + k * CCH
                calls.append((p, w, c0, min(CCH, pw_start + n_pw - c0)))
    chunk_block = np.asarray(chunk_block, np.int64)
    tot_chunks = len(chunk_block)

    # start/stop flags: first/last chunk per block
    start_flag = np.zeros(tot_chunks, bool)
    stop_flag = np.zeros(tot_chunks, bool)
    first_seen = {}
    last_seen = {}
    for i, b in enumerate(chunk_block):
        if b not in first_seen:
            first_seen[b] = i
        last_seen[b] = i
    for b, i in first_seen.items():
        start_flag[i] = True
    for b, i in last_seen.items():
        stop_flag[i] = True

    # per-core slot arrays
    tot_slots = tot_chunks * 128
    per_core = []
    # per-edge slot: cell start + rank within (core, cell)
    cell_id = s_key  # unique per (core,p,w,b)
    # rank within cell
    cell_first = np.zeros_like(s_key)
    starts = np.searchsorted(s_key, np.arange(NCORES * NPASS * NWIN * NBLK))
    # ranks via grouped arange
    uniq, first_idx, counts = np.unique(s_key, return_index=True, return_counts=True)
    rank = np.arange(s_key.size) - np.repeat(first_idx, counts)
    pwb = s_key % (NPASS * NWIN * NBLK)
    pp = pwb // (NWIN * NBLK)
    ww = (pwb // NBLK) % NWIN
    bb = pwb % NBLK
    slot = cell_chunk_start[pp, ww, bb] * 128 + rank
    idxv = (s_src & (WIN - 1)).astype(np.int16)
    dstv = (s_dst & 127).astype(np.float16)
    edge_core = s_key // (NPASS * NWIN * NBLK)
    n_idx_cols = sum(cch * 8 for (_p, _w, _c0, cch) in calls)
    for c in range(NCORES):
        m = edge_core == c
        idx_arr = np.zeros(tot_slots, np.int16)
        dstv_arr = np.full(tot_slots, -1, np.float16)
        idx_arr[slot[m]] = idxv[m]
        dstv_arr[slot[m]] = dstv[m]
        blocks16 = []
        for (_cp, _cw, c0, cch) in calls:
            seg = idx_arr[c0 * 128:(c0 + cch) * 128]
            blocks16.append(np.tile(seg.reshape(-1, 16).T, (8, 1)))
        idx16 = np.concatenate(blocks16, axis=1)
        dstv16 = np.ascontiguousarray(dstv_arr.reshape(tot_chunks, 128).T)
        per_core.append((np.ascontiguousarray(idx16), dstv16))

    sched = {
        "chunk_block": chunk_block,
        "start": start_flag,
        "stop": stop_flag,
        "calls": calls,
        "tot_chunks": tot_chunks,
        "n_idx_cols": n_idx_cols,
    }
    return sched, per_core


# ------------------------------------------------------------ device program
def _build_program(sched, b1_nonzero, b_nonzero, g2_trivial, b2_zero, repeat=1):
    nc = bacc.Bacc("TRN2", target_bir_lowering=False, num_devices=NCORES,
                   num_swdge_queues=4)

    x_d = nc.dram_tensor("xin", [NPAD, D], F32, kind="ExternalInput")
    deg_d = nc.dram_tensor("deg", [128, NTILE], F32, kind="ExternalInput")
    degmy_d = nc.dram_tensor("degmy", [128, NBLK], F32, kind="ExternalInput")
    xres_d = nc.dram_tensor("xres", [PCN, D], F32, kind="ExternalInput")
    w_d = nc.dram_tensor("wmat", [D, D], F32, kind="ExternalInput")
    g1_d = nc.dram_tensor("g1v", [D, 1], F32, kind="ExternalInput")
    b1_d = nc.dram_tensor("b1v", [1, D], F32, kind="ExternalInput")
    g2_d = nc.dram_tensor("g2v", [1, D], F32, kind="ExternalInput")
    b2_d = nc.dram_tensor("b2v", [1, D], F32, kind="ExternalInput")
    bv_d = nc.dram_tensor("bvec", [1, D], F32, kind="ExternalInput")
    idx_d = nc.dram_tensor("idx16", [128, sched["n_idx_cols"]], I16, kind="ExternalInput")
    dstv_d = nc.dram_tensor("dstv", [128, sched["tot_chunks"]], F16, kind="ExternalInput")
    y_d = nc.dram_tensor("y", [PCN, D], F32, kind="ExternalOutput")

    chunk_block = sched["chunk_block"]
    start_flag = sched["start"]
    stop_flag = sched["stop"]
    calls = sched["calls"]
    call_idx_col = {}
    _c = 0
    for (_p, _w, _c0, _cch) in calls:
        call_idx_col[(_p, _w, _c0)] = _c
        _c += _cch * 8

    def bcast_row(dram):  # [1, D] dram -> partition-broadcast AP
        ap = dram[:]
        return bass.AP(tensor=ap.tensor, offset=ap.offset, ap=[[0, 128], [1, D]])

    with tile.TileContext(nc) as tc:
        with (
            tc.tile_pool(name="singles", bufs=1) as singles,
            tc.tile_pool(name="xin_p", bufs=3) as xin_p,
            tc.tile_pool(name="hout_p", bufs=3) as hout_p,
            tc.tile_pool(name="ph1s", bufs=4) as ph1s,
            tc.tile_pool(name="idx_p", bufs=5) as idx_p,
            tc.tile_pool(name="dstv_p", bufs=5) as dstv_p,
            tc.tile_pool(name="oh_p", bufs=4) as oh_p,
            tc.tile_pool(name="msg_p", bufs=4) as msg_p,
            tc.tile_pool(name="fin_p", bufs=4) as fin_p,
            tc.tile_pool(name="fins", bufs=8) as fins,
            tc.tile_pool(name="acc_ps", bufs=5, space="PSUM") as acc_ps,
            tc.tile_pool(name="mm_ps", bufs=2, space="PSUM") as mm_ps,
            tc.tile_pool(name="dram_p", bufs=1, space="DRAM") as dram_p,
        ):
            # ---------------- constants
            iota_i = singles.tile([128, 128], I32)
            nc.gpsimd.iota(iota_i[:], pattern=[[1, 128]], base=0, channel_multiplier=0)
            iota_f = singles.tile([128, 128], F32)
            nc.vector.tensor_copy(out=iota_f[:], in_=iota_i[:])
            iota_h = singles.tile([128, 128], F16)
            nc.vector.tensor_copy(out=iota_h[:], in_=iota_f[:])
            zero512h = singles.tile([128, 512], F16)
            nc.vector.memset(zero512h[:], 0.0)

            w_sb = singles.tile([D, D], F32)
            nc.sync.dma_start(out=w_sb[:], in_=w_d[:])
            g1c = singles.tile([D, 1], F32)
            nc.sync.dma_start(out=g1c[:], in_=g1_d[:])
            # W' = g1[:,None] * W   (folds LN1 gamma into the weight matrix)
            wp_sb = singles.tile([D, D], F32)
            nc.vector.tensor_scalar_mul(out=wp_sb[:], in0=w_sb[:], scalar1=g1c[:])

            if b1_nonzero:
                b1_sb = singles.tile([128, D], F32)
                nc.sync.dma_start(out=b1_sb[:], in_=bcast_row(b1_d))
            if b_nonzero:
                bv_sb = singles.tile([128, D], F32)
                nc.sync.dma_start(out=bv_sb[:], in_=bcast_row(bv_d))
            if not g2_trivial:
                g2_sb = singles.tile([128, D], F32)
                nc.sync.dma_start(out=g2_sb[:], in_=bcast_row(g2_d))
            if not b2_zero:
                b2_sb = singles.tile([128, D], F32)
                nc.sync.dma_start(out=b2_sb[:], in_=bcast_row(b2_d))

            eps_sb = singles.tile([128, 1], F32)
            nc.vector.memset(eps_sb[:], EPS)
            zero512 = singles.tile([128, 512], F32)
            nc.vector.memset(zero512[:], 0.0)
            deg_sb = singles.tile([128, NTILE], F32)
            nc.sync.dma_start(out=deg_sb[:], in_=deg_d[:])
            dinv_sb = singles.tile([128, NTILE], F32)
            nc.scalar.activation(out=dinv_sb[:], in_=deg_sb[:],
                                 func=mybir.ActivationFunctionType.Sqrt)
            nc.vector.reciprocal(out=dinv_sb[:], in_=dinv_sb[:])

            degmy_sb = singles.tile([128, NBLK], F32)
            nc.sync.dma_start(out=degmy_sb[:], in_=degmy_d[:])
            dinvmy_sb = singles.tile([128, NBLK], F32)
            nc.scalar.activation(out=dinvmy_sb[:], in_=degmy_sb[:],
                                 func=mybir.ActivationFunctionType.Sqrt)
            nc.vector.reciprocal(out=dinvmy_sb[:], in_=dinvmy_sb[:])

            for rep in range(repeat):
                h_ws = [
                    dram_p.tile([min(WIN, NPAD - w * WIN), D], F16,
                                tag=f"hw{w}", name=f"h_w{w}_r{rep}")
                    for w in range(NWIN)
                ]
                x_view = x_d[:].rearrange("(t p) f -> p t f", p=128)
                h_views = [
                    h_ws[w][:].rearrange("(t p) f -> p t f", p=128)
                    for w in range(NWIN)
                ]
                TPW = WIN // 128  # x tiles per window (256)
                xres_view = xres_d[:].rearrange("(b p) f -> p b f", p=128)
                y_view = y_d[:].rearrange("(b p) f -> p b f", p=128)

                # ---------------- phase 1: h = dinv * (LN(x) * g1 (+ b1))
                inv_d = 1.0 / D
                for t0 in range(0, NTILE, T_CHUNK):
                    g = min(T_CHUNK, NTILE - t0)
                    xt = xin_p.tile([128, T_CHUNK, D], F32)
                    nc.sync.dma_start(out=xt[:, :g, :], in_=x_view[:, t0:t0 + g, :])
                    sum_t = ph1s.tile([128, T_CHUNK], F32)
                    nc.vector.tensor_reduce(
                        out=sum_t[:, :g], in_=xt[:, :g, :],
                        axis=mybir.AxisListType.X, op=mybir.AluOpType.add)
                    sq = xin_p.tile([128, T_CHUNK, D], F32, tag="sqtile")
                    nc.scalar.activation(out=sq[:, :g, :], in_=xt[:, :g, :],
                                         func=mybir.ActivationFunctionType.Square)
                    ssq_t = ph1s.tile([128, T_CHUNK], F32)
                    nc.vector.tensor_reduce(
                        out=ssq_t[:, :g], in_=sq[:, :g, :],
                        axis=mybir.AxisListType.X, op=mybir.AluOpType.add)
                    mu = ph1s.tile([128, T_CHUNK], F32)
                    nc.scalar.mul(out=mu[:, :g], in_=sum_t[:, :g], mul=inv_d)
                    var = ph1s.tile([128, T_CHUNK], F32)
                    nc.scalar.mul(out=var[:, :g], in_=ssq_t[:, :g], mul=inv_d)
                    mu2 = ph1s.tile([128, T_CHUNK], F32)
                    nc.vector.tensor_mul(out=mu2[:, :g], in0=mu[:, :g], in1=mu[:, :g])
                    nc.vector.tensor_sub(out=var[:, :g], in0=var[:, :g], in1=mu2[:, :g])
                    nc.scalar.activation(out=var[:, :g], in_=var[:, :g],
                                         func=mybir.ActivationFunctionType.Sqrt,
                                         bias=eps_sb[:])
                    rstd = ph1s.tile([128, T_CHUNK], F32)
                    nc.vector.reciprocal(out=rstd[:, :g], in_=var[:, :g])
                    # fold dinv into rstd; bias = -mu * rstd'
                    nc.vector.tensor_mul(out=rstd[:, :g], in0=rstd[:, :g],
                                         in1=dinv_sb[:, t0:t0 + g])
                    nmu = ph1s.tile([128, T_CHUNK], F32)
                    nc.vector.tensor_mul(out=nmu[:, :g], in0=mu[:, :g], in1=rstd[:, :g])
                    nc.scalar.mul(out=nmu[:, :g], in_=nmu[:, :g], mul=-1.0)
                    htmp = hout_p.tile([128, T_CHUNK, D], F32, tag="htmp")
                    nc.vector.tensor_tensor(
                        out=htmp[:, :g, :], in0=xt[:, :g, :],
                        in1=rstd[:, :g].to_broadcast([128, g, D]),
                        op=mybir.AluOpType.mult)
                    ht = hout_p.tile([128, T_CHUNK, D], F16)
                    nc.vector.tensor_tensor(
                        out=ht[:, :g, :], in0=htmp[:, :g, :],
                        in1=nmu[:, :g].to_broadcast([128, g, D]),
                        op=mybir.AluOpType.add)
                    if b1_nonzero:
                        # h += dinv * b1  (rare path; b1 is zero in this problem)
                        for j in range(g):
                            tmp = ph1s.tile([128, D], F32, tag="b1tmp")
                            nc.vector.tensor_scalar_mul(
                                out=tmp[:], in0=b1_sb[:],
                                scalar1=dinv_sb[:, t0 + j:t0 + j + 1])
                            nc.vector.tensor_add(out=ht[:, j, :], in0=ht[:, j, :], in1=tmp[:])
                    wh = t0 // TPW
                    nc.sync.dma_start(
                        out=h_views[wh][:, t0 - wh * TPW:t0 - wh * TPW + g, :],
                        in_=ht[:, :g, :])

                # ---------------- phase 2: gather + one-hot matmul scatter
                idx_col = 0
                for p in range(NPASS):
                    accs = [
                        acc_ps.tile([128, 512], F32, tag="accbank", name=f"acc_r{rep}_p{p}_{i}")
                        for i in range(5)
                    ]
                    # start=True resets the whole PSUM bank, so initialize each
                    # bank exactly once and accumulate all chunks with start=False
                    for acc in accs:
                        nc.tensor.matmul(out=acc[:], lhsT=iota_h[:], rhs=zero512h[:],
                                         start=True, stop=False, skip_group_check=True)
                    qrr = 0
                    for (cp, cw, chunk0, cch) in calls:
                        if cp != p:
                            if cp < p:
                                pass
                            continue
                        w = cw
                        S = cch * 8
                        ic0 = call_idx_col[(cp, cw, chunk0)]
                        idx_t = idx_p.tile([128, CCH * 8], I16)
                        nc.sync.dma_start(out=idx_t[:, :S], in_=idx_d[:, ic0:ic0 + S])
                        dstv_t = dstv_p.tile([128, CCH], F16)
                        nc.sync.dma_start(out=dstv_t[:, :cch], in_=dstv_d[:, chunk0:chunk0 + cch])
                        oh = oh_p.tile([128, CCH, 128], F16)
                        iap = iota_h[:]
                        nc.vector.tensor_tensor(
                            out=oh[:, :cch, :],
                            in0=dstv_t[:, :cch].to_broadcast([128, cch, 128]),
                            in1=bass.AP(tensor=iap.tensor, offset=iap.offset,
                                        ap=[list(iap.ap[0]), [0, cch], list(iap.ap[1])]),
                            op=mybir.AluOpType.is_equal)
                        msg = msg_p.tile([128, CCH, D], F16)
                        nc.gpsimd.dma_gather(
                            out_ap=msg[:, :cch, :],
                            in_ap=h_ws[w][:],
                            idxs_ap=idx_t[:, :S],
                            num_idxs=cch * 128, num_idxs_reg=cch * 128,
                            elem_size=D, single_packet=False,
                            queue_num=qrr % 4)
                        qrr += 1
                        for j in range(cch):
                            gi = chunk0 + j
                            b = int(chunk_block[gi])
                            a = b - PASS_START[p]
                            nc.tensor.matmul(
                                out=accs[a // 4][:, (a % 4) * 128:(a % 4) * 128 + 128],
                                lhsT=msg[:, j, :],
                                rhs=oh[:, j, :],
                                start=False,
                                stop=bool(stop_flag[gi]),
                                skip_group_check=True)

                    # ---------------- phase 3: finalize pass p (bank-batched)
                    p_end = PASS_START[p] + PASS_BLOCKS[p]
                    for b0 in range(PASS_START[p], p_end, 4):
                        g = min(4, p_end - b0)
                        bank = (b0 - PASS_START[p]) // 4
                        accS = fin_p.tile([128, 512], F32, tag="accS")
                        nc.vector.tensor_copy(out=accS[:, :g * 128],
                                              in_=accs[bank][:, :g * 128])
                        mm_bank = mm_ps.tile([128, 512], F32, tag="mmbank")
                        nc.tensor.matmul(out=mm_bank[:], lhsT=iota_f[:],
                                         rhs=zero512[:], start=True, stop=False,
                                         skip_group_check=True)
                        for j in range(g):
                            nc.tensor.matmul(
                                out=mm_bank[:, j * 128:(j + 1) * 128],
                                lhsT=accS[:, j * 128:(j + 1) * 128],
                                rhs=wp_sb[:], start=False, stop=True,
                                skip_group_check=True)
                        o1 = fin_p.tile([128, 4, 128], F32, tag="o1")
                        nc.vector.tensor_tensor(
                            out=o1[:, :g, :],
                            in0=mm_bank[:].rearrange("p (b f) -> p b f", b=4)[:, :g, :],
                            in1=dinvmy_sb[:, b0:b0 + g].to_broadcast([128, g, 128]),
                            op=mybir.AluOpType.mult)
                        if b_nonzero:
                            for j in range(g):
                                nc.vector.tensor_add(out=o1[:, j, :],
                                                     in0=o1[:, j, :], in1=bv_sb[:])
                        xr = fin_p.tile([128, 4, 128], F32, tag="xr")
                        nc.sync.dma_start(out=xr[:, :g, :],
                                          in_=xres_view[:, b0:b0 + g, :])
                        nc.vector.tensor_add(out=o1[:, :g, :], in0=o1[:, :g, :],
                                             in1=xr[:, :g, :])
                        # LN2, batched stats over g blocks
                        sum2 = fins.tile([128, 4], F32, tag="sum2")
                        nc.vector.tensor_reduce(out=sum2[:, :g], in_=o1[:, :g, :],
                                                axis=mybir.AxisListType.X,
                                                op=mybir.AluOpType.add)
                        sq2 = fin_p.tile([128, 4, 128], F32, tag="sq2")
                        nc.scalar.activation(out=sq2[:, :g, :], in_=o1[:, :g, :],
                                             func=mybir.ActivationFunctionType.Square)
                        ssq2 = fins.tile([128, 4], F32, tag="ssq2")
                        nc.vector.tensor_reduce(out=ssq2[:, :g], in_=sq2[:, :g, :],
                                                axis=mybir.AxisListType.X,
                                                op=mybir.AluOpType.add)
                        mu_2 = fins.tile([128, 4], F32, tag="mu2f")
                        nc.scalar.mul(out=mu_2[:, :g], in_=sum2[:, :g], mul=inv_d)
                        var2 = fins.tile([128, 4], F32, tag="var2")
                        nc.scalar.mul(out=var2[:, :g], in_=ssq2[:, :g], mul=inv_d)
                        mu22 = fins.tile([128, 4], F32, tag="mu22")
                        nc.vector.tensor_mul(out=mu22[:, :g], in0=mu_2[:, :g],
                                             in1=mu_2[:, :g])
                        nc.vector.tensor_sub(out=var2[:, :g], in0=var2[:, :g],
                                             in1=mu22[:, :g])
                        nc.scalar.activation(out=var2[:, :g], in_=var2[:, :g],
                                             func=mybir.ActivationFunctionType.Sqrt,
                                             bias=eps_sb[:])
                        rstd2 = fins.tile([128, 4], F32, tag="rstd2")
                        nc.vector.reciprocal(out=rstd2[:, :g], in_=var2[:, :g])
                        nmu2 = fins.tile([128, 4], F32, tag="nmu2")
                        nc.vector.tensor_mul(out=nmu2[:, :g], in0=mu_2[:, :g],
                                             in1=rstd2[:, :g])
                        nc.scalar.mul(out=nmu2[:, :g], in_=nmu2[:, :g], mul=-1.0)
                        yt = fin_p.tile([128, 4, 128], F32, tag="yt")
                        nc.vector.tensor_tensor(
                            out=yt[:, :g, :], in0=o1[:, :g, :],
                            in1=rstd2[:, :g].to_broadcast([128, g, 128]),
                            op=mybir.AluOpType.mult)
                        nc.vector.tensor_tensor(
                            out=yt[:, :g, :], in0=yt[:, :g, :],
                            in1=nmu2[:, :g].to_broadcast([128, g, 128]),
                            op=mybir.AluOpType.add)
                        if not g2_trivial:
                            for j in range(g):
                                nc.vector.tensor_mul(out=yt[:, j, :],
                                                     in0=yt[:, j, :], in1=g2_sb[:])
                        if not b2_zero:
                            for j in range(g):
                                nc.vector.tensor_add(out=yt[:, j, :],
                                                     in0=yt[:, j, :], in1=b2_sb[:])
                        nc.sync.dma_start(out=y_view[:, b0:b0 + g, :],
                                          in_=yt[:, :g, :])
    nc.compile()
    _split_excess_waits(nc, max_waits=1)
    return nc


_CACHE = {}


def _get_program_and_data(x, edge_index, W, b, g1, b1, g2, b2, repeat=1):
    src = np.asarray(edge_index[0], np.int64)
    dst = np.asarray(edge_index[1], np.int64)
    loops = np.arange(N, dtype=np.int64)
    src = np.concatenate([src, loops])
    dst = np.concatenate([dst, loops])

    deg = np.bincount(dst, minlength=NPAD).astype(np.float32)
    deg[deg == 0] = 1.0

    sched, per_core = _build_schedule(src, dst)

    b1_nonzero = bool(np.any(b1))
    b_nonzero = bool(np.any(b))
    g2_trivial = bool(np.all(g2 == 1.0))
    b2_zero = not bool(np.any(b2))

    nc = _build_program(sched, b1_nonzero, b_nonzero, g2_trivial, b2_zero, repeat=repeat)

    x_pad = np.zeros((NPAD, D), np.float32)
    x_pad[:N] = np.asarray(x, np.float32)
    deg_arr = np.ascontiguousarray(deg.reshape(NTILE, 128).T)

    in_maps = []
    for c in range(NCORES):
        idx16, dstv16 = per_core[c]
        degmy = np.ascontiguousarray(
            deg[c * PCN:(c + 1) * PCN].reshape(NBLK, 128).T)
        in_maps.append({
            "xin": x_pad,
            "deg": deg_arr,
            "degmy": degmy,
            "xres": x_pad[c * PCN:(c + 1) * PCN],
            "wmat": np.ascontiguousarray(np.asarray(W, np.float32)),
            "g1v": np.asarray(g1, np.float32).reshape(D, 1),
            "b1v": np.asarray(b1, np.float32).reshape(1, D),
            "g2v": np.asarray(g2, np.float32).reshape(1, D),
            "b2v": np.asarray(b2, np.float32).reshape(1, D),
            "bvec": np.asarray(b, np.float32).reshape(1, D),
            "idx16": idx16,
            "dstv": dstv16,
        })
    return nc, in_maps


def kernel(x, edge_index, W, b, g1, b1, g2, b2):
    nc, in_maps = _get_program_and_data(x, edge_index, W, b, g1, b1, g2, b2)
    res = run_bass_kernel_spmd(nc, in_maps, core_ids=list(range(NCORES)))
    out = np.concatenate([res.results[c]["y"] for c in range(NCORES)], axis=0)
    return out[:N].astype(np.float32)


if __name__ == "__main__":
    # tiny self-check of the host schedule against numpy
    rng = np.random.default_rng(0)
    E = 3200000
    src = rng.integers(0, N, E).astype(np.int64)
    dst = rng.integers(0, N, E).astype(np.int64)
    loops = np.arange(N, dtype=np.int64)
    srcl = np.concatenate([src, loops])
    dstl = np.concatenate([dst, loops])
    sched, per_core = _build_schedule(srcl, dstl)
    print("tot chunks:", sched["tot_chunks"], "calls:", len(sched["calls"]))
    print("pad overhead:", sched["tot_chunks"] * 128 / len(srcl) - 1)

